# revision 13
# baseline (speedup 1.0000x reference)
"""Trainium2 Bass kernel for DiffusionCoordinateInitializer.

Reference computation:
    coords = einsum("bsd,cd->bsc", latent, W) + b          # [B, S, 3]
    x = noise; for t in reversed(range(T)): x = a*x + (1-a)*coords, a=(t+1)/T
which collapses (affine fixed-point iteration) to
    x = A*noise + (1-A)*(coords + b),  A = prod_{t=1..T} t/T = T!/T^T

Strategy (pure data-parallel over 8 cores, token-sharded):
  - Host folds (1-A) into W^T and A*noise + (1-A)*b into a bias tensor, so
    the device computes out^T[3, tok] = (W_eff @ latent^T) + bias^T.
  - Host pre-transposes + downcasts latent to fp16 [d, tok] per core, so the
    device streams contraction-major tiles straight into accumulating
    matmuls: no on-chip transposes (v5's PE bottleneck) and half the HBM
    traffic (DMA floor ~47 us/core instead of ~94 us).
  - Chunk-major schedule: for each 128-row d-chunk, one [128, 4096] fp16
    load feeds 8 skinny matmuls (W chunk stationary [128, 3], moving
    [128, 512]) accumulating into 8 PSUM banks, one per 512-token super.
  - DVE adds the bias tensor out of PSUM; one [3, 4096] store per core.
"""

import numpy as np
from contextlib import ExitStack

import concourse.bass as bass  # noqa: F401
import concourse.tile as tile
from concourse import bacc, mybir
from concourse.bass_utils import run_bass_kernel_spmd

N_CORES = 8
B, S, D = 4, 8192, 2048
TOK = B * S                      # 32768
TPC = TOK // N_CORES             # 4096 tokens per core
P = 128
SUPER = 512                      # tokens per PSUM bank (max psum free f32)
N_SUPER = TPC // SUPER           # 8
N_CHUNK = D // P                 # 16
F32 = mybir.dt.float32
F16 = mybir.dt.float16
F8E3 = mybir.dt.float8e3

_NC_CACHE = {}


def _build_nc_v7(lat_dt=F16, repeat=1):
    """Pre-transposed stream: latT [D, TPC] (fp16 or fp8e3m4) in DRAM,
    chunk-major accumulating matmuls into 8 PSUM banks, no transposes."""
    key = ("v7", lat_dt, repeat)
    if key in _NC_CACHE:
        return _NC_CACHE[key]

    nc = bacc.Bacc("TRN2", target_bir_lowering=False, debug=False,
                   enable_asserts=False, num_devices=N_CORES)
    latT = nc.dram_tensor("latT", [D, TPC], lat_dt, kind="ExternalInput").ap()
    # host prepacks W_eff^T chunks as [128, 16*3]: wt[p, 3k+c] = W_eff[c, 128k+p]
    wt = nc.dram_tensor("wt", [P, 3 * N_CHUNK], F16, kind="ExternalInput").ap()
    nzt = nc.dram_tensor("nzt", [3, TPC], F32, kind="ExternalInput").ap()
    out = nc.dram_tensor("out", [3, TPC], F32, kind="ExternalOutput").ap()

    with tile.TileContext(nc) as tc:
        with ExitStack() as ctx:
            const = ctx.enter_context(tc.tile_pool(name="const", bufs=1))
            lat_pool = ctx.enter_context(tc.tile_pool(name="lat", bufs=4))
            ps_pool = ctx.enter_context(tc.tile_pool(name="ps", bufs=1, space="PSUM"))
            osb_pool = ctx.enter_context(tc.tile_pool(name="osb", bufs=2))

            wt_t = const.tile([P, 3 * N_CHUNK], F16)
            nc.sync.dma_start(wt_t[:], wt[:])
            nz_t = const.tile([3, TPC], F32)
            nc.sync.dma_start(nz_t[:], nzt[:])

            for _ in range(repeat):
                pss = [ps_pool.tile([3, SUPER], F32, name=f"ps{s}", tag=f"ps{s}")
                       for s in range(N_SUPER)]
                for k in range(N_CHUNK):
                    lt = lat_pool.tile([P, TPC], F16, name="lt", tag="lt")
                    nc.sync.dma_start(lt[:], latT[k * P:(k + 1) * P, :])
                    for s in range(N_SUPER):
                        nc.tensor.matmul(
                            pss[s][:], wt_t[:, k * 3:(k + 1) * 3],
                            lt[:, s * SUPER:(s + 1) * SUPER],
                            start=(k == 0), stop=(k == N_CHUNK - 1),
                        )
                osb = osb_pool.tile([3, TPC], F32, name="osb", tag="osb")
                for s in range(N_SUPER):
                    nc.vector.tensor_add(osb[:, s * SUPER:(s + 1) * SUPER],
                                         pss[s][:], nz_t[:, s * SUPER:(s + 1) * SUPER])
                nc.sync.dma_start(out[:], osb[:])

    nc.compile()
    _NC_CACHE[key] = nc
    return nc


PIECE = 1024                     # tokens per DMA piece (2 KB/part fp16)
N_PIECE = TPC // PIECE           # 4 pieces per chunk


def _build_nc_v8(lat_dt=F16, wt_dt=F16, repeat=1):
    """Piece-granular stream + interleaved drain.

    Same math as v7 but: each 128-row d-chunk is loaded as 4 [128, 1024]
    pieces so the first matmul starts ~8 us earlier; after the last chunk,
    each super's bias-add runs on alternating Vector/Scalar engines right
    behind its stop-matmul, and its [3, 512] store issues immediately --
    the drain hides under the PE tail instead of serializing after it.
    """
    key = ("v8", lat_dt, wt_dt, repeat)
    if key in _NC_CACHE:
        return _NC_CACHE[key]

    nc = bacc.Bacc("TRN2", target_bir_lowering=False, debug=False,
                   enable_asserts=False, num_devices=N_CORES)
    latT = nc.dram_tensor("latT", [D, TPC], lat_dt, kind="ExternalInput").ap()
    wt = nc.dram_tensor("wt", [P, 3 * N_CHUNK], wt_dt, kind="ExternalInput").ap()
    nzt = nc.dram_tensor("nzt", [3, TPC], F32, kind="ExternalInput").ap()
    out = nc.dram_tensor("out", [3, TPC], F32, kind="ExternalOutput").ap()

    SPP = PIECE // SUPER  # supers per piece (2)

    with tile.TileContext(nc) as tc:
        with ExitStack() as ctx:
            const = ctx.enter_context(tc.tile_pool(name="const", bufs=1))
            lat_pool = ctx.enter_context(tc.tile_pool(name="lat", bufs=6))
            ps_pool = ctx.enter_context(tc.tile_pool(name="ps", bufs=1, space="PSUM"))
            osb_pool = ctx.enter_context(tc.tile_pool(name="osb", bufs=8))

            # consts via engine sequencers: the Sync sequencer spends the
            # first ~9 us on queue init, and a DIRECT2D issued there would
            # gate the first matmul on the weights until ~11 us.
            wt_t = const.tile([P, 3 * N_CHUNK], wt_dt)
            nc.scalar.dma_start(wt_t[:], wt[:])
            nz_t = const.tile([3, TPC], F32)
            nc.gpsimd.dma_start(nz_t[:], nzt[:])

            for _ in range(repeat):
                pss = [ps_pool.tile([3, SUPER], F32, name=f"ps{s}", tag=f"ps{s}")
                       for s in range(N_SUPER)]
                for k in range(N_CHUNK):
                    # chunk 0 in 512-token pieces so the first matmul's
                    # dependency lands ~2 us after DMA start; 1024 after
                    w = SUPER if k == 0 else PIECE
                    spp = w // SUPER
                    pieces = []
                    for p in range(TPC // w):
                        lt = lat_pool.tile([P, w], lat_dt, name="lt", tag="lt")
                        nc.sync.dma_start(
                            lt[:], latT[k * P:(k + 1) * P, p * w:(p + 1) * w])
                        pieces.append(lt)
                    for s in range(N_SUPER):
                        nc.tensor.matmul(
                            pss[s][:], wt_t[:, k * 3:(k + 1) * 3],
                            pieces[s // spp][:, (s % spp) * SUPER:
                                             (s % spp + 1) * SUPER],
                            start=(k == 0), stop=(k == N_CHUNK - 1),
                        )
                        if k == N_CHUNK - 1:
                            osb = osb_pool.tile([3, SUPER], F32,
                                                name="osb", tag="osb")
                            nc.vector.tensor_add(osb[:], pss[s][:],
                                                 nz_t[:, s * SUPER:(s + 1) * SUPER])
                            nc.scalar.dma_start(
                                out[:, s * SUPER:(s + 1) * SUPER], osb[:])

    nc.compile()
    _NC_CACHE[key] = nc
    return nc


def _build_nc_v9(lat_dt=F16, wt_dt=F16, repeat=1):
    """v8 + bias-add folded into the PE and stores straight from PSUM.

    The noise/bias term enters each super's accumulation group as one extra
    matmul: stationary = I3 [3, 3], moving = nz16 [3, 512] fp16, so
    psum += I3^T @ nz = nz elementwise. No Vector/Scalar engine work at
    all; each super's [3, 512] result DMAs from PSUM as soon as its group
    stops, hiding the whole drain under the PE tail.
    """
    key = ("v9", lat_dt, wt_dt, repeat)
    if key in _NC_CACHE:
        return _NC_CACHE[key]

    nc = bacc.Bacc("TRN2", target_bir_lowering=False, debug=False,
                   enable_asserts=False, num_devices=N_CORES)
    latT = nc.dram_tensor("latT", [D, TPC], lat_dt, kind="ExternalInput").ap()
    wt = nc.dram_tensor("wt", [P, 3 * N_CHUNK], wt_dt, kind="ExternalInput").ap()
    ident3 = nc.dram_tensor("ident3", [3, 3], F16, kind="ExternalInput").ap()
    nzt = nc.dram_tensor("nzt", [3, TPC], F16, kind="ExternalInput").ap()
    out = nc.dram_tensor("out", [3, TPC], F32, kind="ExternalOutput").ap()

    SPP = PIECE // SUPER  # supers per piece (2)

    with tile.TileContext(nc) as tc:
        with ExitStack() as ctx:
            const = ctx.enter_context(tc.tile_pool(name="const", bufs=1))
            lat_pool = ctx.enter_context(tc.tile_pool(name="lat", bufs=12))
            ps_pool = ctx.enter_context(tc.tile_pool(name="ps", bufs=1, space="PSUM"))

            wt_t = const.tile([P, 3 * N_CHUNK], wt_dt)
            nc.sync.dma_start(wt_t[:], wt[:])
            id3_t = const.tile([3, 3], F16)
            nc.sync.dma_start(id3_t[:], ident3[:])
            nz_t = const.tile([3, TPC], F16)
            nc.sync.dma_start(nz_t[:], nzt[:])

            for _ in range(repeat):
                pss = [ps_pool.tile([3, SUPER], F32, name=f"ps{s}", tag=f"ps{s}")
                       for s in range(N_SUPER)]
                for k in range(N_CHUNK):
                    pieces = []
                    for p in range(N_PIECE):
                        lt = lat_pool.tile([P, PIECE], lat_dt, name="lt", tag="lt")
                        nc.sync.dma_start(
                            lt[:], latT[k * P:(k + 1) * P,
                                        p * PIECE:(p + 1) * PIECE])
                        pieces.append(lt)
                    for s in range(N_SUPER):
                        nc.tensor.matmul(
                            pss[s][:], wt_t[:, k * 3:(k + 1) * 3],
                            pieces[s // SPP][:, (s % SPP) * SUPER:
                                             (s % SPP + 1) * SUPER],
                            start=(k == 0), stop=False,
                        )
                        if k == N_CHUNK - 1:
                            nc.tensor.matmul(
                                pss[s][:], id3_t[:],
                                nz_t[:, s * SUPER:(s + 1) * SUPER],
                                start=False, stop=True,
                            )
                            nc.sync.dma_start(
                                out[:, s * SUPER:(s + 1) * SUPER], pss[s][:])

    nc.compile()
    _NC_CACHE[key] = nc
    return nc


def _coeff(T: int) -> float:
    a = 1.0
    for t in range(T):
        a *= (t + 1) / T
    return a


PIPELINE = "v8_fp8"  # "v7" | "v8_fp16" | "v8_fp8" | "v9_fp16" | "v9_fp8"


def kernel(latent, W, b, noise, diffusion_steps, _trace=False, _pipeline=None):
    import ml_dtypes
    T = int(diffusion_steps)
    A = _coeff(T)
    pipeline = _pipeline or PIPELINE
    fp8 = pipeline.endswith("fp8")
    v9 = pipeline.startswith("v9")

    lat_flat = np.ascontiguousarray(latent.reshape(TOK, D), dtype=np.float32)
    if fp8:
        latT_h = lat_flat.astype(ml_dtypes.float8_e3m4).T  # [D, TOK] view
    else:
        latT_h = lat_flat.astype(np.float16).T
    wt_eff = np.ascontiguousarray(W.T).astype(np.float32) * np.float32(1.0 - A)
    # prepack [2048, 3] -> [128, 16*3]: chunk k (rows 128k..128k+128) at cols 3k..3k+3
    wt_packed = np.ascontiguousarray(
        wt_eff.reshape(N_CHUNK, P, 3).transpose(1, 0, 2).reshape(P, 3 * N_CHUNK)
    ).astype(np.float16)
    nz_eff = (np.float32(A) * noise.reshape(TOK, 3)
              + np.float32(1.0 - A) * b[None, :].astype(np.float32))
    nz_dt = np.float16 if v9 else np.float32
    nz_eff_t = np.ascontiguousarray(nz_eff.T.astype(nz_dt))  # [3, TOK]

    lat_dt = mybir.dt.float8e3 if fp8 else F16
    if pipeline == "v7":
        nc = _build_nc_v7()
    elif v9:
        nc = _build_nc_v9(lat_dt=lat_dt)
    else:
        nc = _build_nc_v8(lat_dt=lat_dt)
    in_maps = []
    for c in range(N_CORES):
        im = {
            "latT": np.ascontiguousarray(latT_h[:, c * TPC:(c + 1) * TPC]),
            "wt": wt_packed,
            "nzt": np.ascontiguousarray(nz_eff_t[:, c * TPC:(c + 1) * TPC]),
        }
        if v9:
            im["ident3"] = np.eye(3, dtype=np.float16)
        in_maps.append(im)
    res = run_bass_kernel_spmd(nc, in_maps, core_ids=list(range(N_CORES)),
                               trace=_trace)
    out = np.empty((TOK, 3), dtype=np.float32)
    for c in range(N_CORES):
        out[c * TPC:(c + 1) * TPC] = res.results[c]["out"].T
    if _trace:
        kernel._last_results = res
    return out.reshape(B, S, 3)


# revision 15
# speedup vs baseline: 1.0259x; 1.0259x over previous
"""Trainium2 Bass kernel for DiffusionCoordinateInitializer.

Reference computation:
    coords = einsum("bsd,cd->bsc", latent, W) + b          # [B, S, 3]
    x = noise; for t in reversed(range(T)): x = a*x + (1-a)*coords, a=(t+1)/T
which collapses (affine fixed-point iteration) to
    x = A*noise + (1-A)*(coords + b),  A = prod_{t=1..T} t/T = T!/T^T

Strategy (pure data-parallel over 8 cores, token-sharded):
  - Host folds (1-A) into W^T and A*noise + (1-A)*b into a bias tensor, so
    the device computes out^T[3, tok] = (W_eff @ latent^T) + bias^T.
  - Host pre-transposes + downcasts latent to fp16 [d, tok] per core, so the
    device streams contraction-major tiles straight into accumulating
    matmuls: no on-chip transposes (v5's PE bottleneck) and half the HBM
    traffic (DMA floor ~47 us/core instead of ~94 us).
  - Chunk-major schedule: for each 128-row d-chunk, one [128, 4096] fp16
    load feeds 8 skinny matmuls (W chunk stationary [128, 3], moving
    [128, 512]) accumulating into 8 PSUM banks, one per 512-token super.
  - DVE adds the bias tensor out of PSUM; one [3, 4096] store per core.
"""

import numpy as np
from contextlib import ExitStack

import concourse.bass as bass  # noqa: F401
import concourse.tile as tile
from concourse import bacc, mybir
from concourse.bass_utils import run_bass_kernel_spmd

N_CORES = 8
B, S, D = 4, 8192, 2048
TOK = B * S                      # 32768
TPC = TOK // N_CORES             # 4096 tokens per core
P = 128
SUPER = 512                      # tokens per PSUM bank (max psum free f32)
N_SUPER = TPC // SUPER           # 8
N_CHUNK = D // P                 # 16
F32 = mybir.dt.float32
F16 = mybir.dt.float16
F8E3 = mybir.dt.float8e3

_NC_CACHE = {}


def _build_nc_v7(lat_dt=F16, repeat=1):
    """Pre-transposed stream: latT [D, TPC] (fp16 or fp8e3m4) in DRAM,
    chunk-major accumulating matmuls into 8 PSUM banks, no transposes."""
    key = ("v7", lat_dt, repeat)
    if key in _NC_CACHE:
        return _NC_CACHE[key]

    nc = bacc.Bacc("TRN2", target_bir_lowering=False, debug=False,
                   enable_asserts=False, num_devices=N_CORES)
    latT = nc.dram_tensor("latT", [D, TPC], lat_dt, kind="ExternalInput").ap()
    # host prepacks W_eff^T chunks as [128, 16*3]: wt[p, 3k+c] = W_eff[c, 128k+p]
    wt = nc.dram_tensor("wt", [P, 3 * N_CHUNK], F16, kind="ExternalInput").ap()
    nzt = nc.dram_tensor("nzt", [3, TPC], F32, kind="ExternalInput").ap()
    out = nc.dram_tensor("out", [3, TPC], F32, kind="ExternalOutput").ap()

    with tile.TileContext(nc) as tc:
        with ExitStack() as ctx:
            const = ctx.enter_context(tc.tile_pool(name="const", bufs=1))
            lat_pool = ctx.enter_context(tc.tile_pool(name="lat", bufs=4))
            ps_pool = ctx.enter_context(tc.tile_pool(name="ps", bufs=1, space="PSUM"))
            osb_pool = ctx.enter_context(tc.tile_pool(name="osb", bufs=2))

            wt_t = const.tile([P, 3 * N_CHUNK], F16)
            nc.sync.dma_start(wt_t[:], wt[:])
            nz_t = const.tile([3, TPC], F32)
            nc.sync.dma_start(nz_t[:], nzt[:])

            for _ in range(repeat):
                pss = [ps_pool.tile([3, SUPER], F32, name=f"ps{s}", tag=f"ps{s}")
                       for s in range(N_SUPER)]
                for k in range(N_CHUNK):
                    lt = lat_pool.tile([P, TPC], F16, name="lt", tag="lt")
                    nc.sync.dma_start(lt[:], latT[k * P:(k + 1) * P, :])
                    for s in range(N_SUPER):
                        nc.tensor.matmul(
                            pss[s][:], wt_t[:, k * 3:(k + 1) * 3],
                            lt[:, s * SUPER:(s + 1) * SUPER],
                            start=(k == 0), stop=(k == N_CHUNK - 1),
                        )
                osb = osb_pool.tile([3, TPC], F32, name="osb", tag="osb")
                for s in range(N_SUPER):
                    nc.vector.tensor_add(osb[:, s * SUPER:(s + 1) * SUPER],
                                         pss[s][:], nz_t[:, s * SUPER:(s + 1) * SUPER])
                nc.sync.dma_start(out[:], osb[:])

    nc.compile()
    _NC_CACHE[key] = nc
    return nc


PIECE = 1024                     # tokens per DMA piece (2 KB/part fp16)
N_PIECE = TPC // PIECE           # 4 pieces per chunk


def _build_nc_v8(lat_dt=F16, wt_dt=F16, repeat=1):
    """Piece-granular stream + interleaved drain.

    Same math as v7 but: each 128-row d-chunk is loaded as 4 [128, 1024]
    pieces so the first matmul starts ~8 us earlier; after the last chunk,
    each super's bias-add runs on alternating Vector/Scalar engines right
    behind its stop-matmul, and its [3, 512] store issues immediately --
    the drain hides under the PE tail instead of serializing after it.
    """
    key = ("v8", lat_dt, wt_dt, repeat)
    if key in _NC_CACHE:
        return _NC_CACHE[key]

    nc = bacc.Bacc("TRN2", target_bir_lowering=False, debug=False,
                   enable_asserts=False, num_devices=N_CORES)
    latT = nc.dram_tensor("latT", [D, TPC], lat_dt, kind="ExternalInput").ap()
    wt = nc.dram_tensor("wt", [P, 3 * N_CHUNK], wt_dt, kind="ExternalInput").ap()
    nzt = nc.dram_tensor("nzt", [3, TPC], F32, kind="ExternalInput").ap()
    out = nc.dram_tensor("out", [3, TPC], F32, kind="ExternalOutput").ap()

    SPP = PIECE // SUPER  # supers per piece (2)

    with tile.TileContext(nc) as tc:
        with ExitStack() as ctx:
            const = ctx.enter_context(tc.tile_pool(name="const", bufs=1))
            lat_pool = ctx.enter_context(tc.tile_pool(name="lat", bufs=16))
            ps_pool = ctx.enter_context(tc.tile_pool(name="ps", bufs=1, space="PSUM"))
            osb_pool = ctx.enter_context(tc.tile_pool(name="osb", bufs=8))

            # consts via engine sequencers: the Sync sequencer spends the
            # first ~9 us on queue init, and a DIRECT2D issued there would
            # gate the first matmul on the weights until ~11 us.
            wt_t = const.tile([P, 3 * N_CHUNK], wt_dt)
            nc.scalar.dma_start(wt_t[:], wt[:])
            nz_t = const.tile([3, TPC], F32)
            nc.gpsimd.dma_start(nz_t[:], nzt[:])

            for _ in range(repeat):
                pss = [ps_pool.tile([3, SUPER], F32, name=f"ps{s}", tag=f"ps{s}")
                       for s in range(N_SUPER)]
                for k in range(N_CHUNK):
                    # chunk 0 in 512-token pieces so the first matmul's
                    # dependency lands ~2 us after DMA start; 1024 after
                    w = SUPER if k == 0 else PIECE
                    spp = w // SUPER
                    pieces = []
                    for p in range(TPC // w):
                        lt = lat_pool.tile([P, w], lat_dt, name="lt", tag="lt")
                        nc.sync.dma_start(
                            lt[:], latT[k * P:(k + 1) * P, p * w:(p + 1) * w])
                        pieces.append(lt)
                    for s in range(N_SUPER):
                        nc.tensor.matmul(
                            pss[s][:], wt_t[:, k * 3:(k + 1) * 3],
                            pieces[s // spp][:, (s % spp) * SUPER:
                                             (s % spp + 1) * SUPER],
                            start=(k == 0), stop=(k == N_CHUNK - 1),
                        )
                        if k == N_CHUNK - 1:
                            osb = osb_pool.tile([3, SUPER], F32,
                                                name="osb", tag="osb")
                            nc.vector.tensor_add(osb[:], pss[s][:],
                                                 nz_t[:, s * SUPER:(s + 1) * SUPER])
                            nc.scalar.dma_start(
                                out[:, s * SUPER:(s + 1) * SUPER], osb[:])

    nc.compile()
    _NC_CACHE[key] = nc
    return nc


def _build_nc_v9(lat_dt=F16, wt_dt=F16, repeat=1):
    """v8 + bias-add folded into the PE and stores straight from PSUM.

    The noise/bias term enters each super's accumulation group as one extra
    matmul: stationary = I3 [3, 3], moving = nz16 [3, 512] fp16, so
    psum += I3^T @ nz = nz elementwise. No Vector/Scalar engine work at
    all; each super's [3, 512] result DMAs from PSUM as soon as its group
    stops, hiding the whole drain under the PE tail.
    """
    key = ("v9", lat_dt, wt_dt, repeat)
    if key in _NC_CACHE:
        return _NC_CACHE[key]

    nc = bacc.Bacc("TRN2", target_bir_lowering=False, debug=False,
                   enable_asserts=False, num_devices=N_CORES)
    latT = nc.dram_tensor("latT", [D, TPC], lat_dt, kind="ExternalInput").ap()
    wt = nc.dram_tensor("wt", [P, 3 * N_CHUNK], wt_dt, kind="ExternalInput").ap()
    ident3 = nc.dram_tensor("ident3", [3, 3], F16, kind="ExternalInput").ap()
    nzt = nc.dram_tensor("nzt", [3, TPC], F16, kind="ExternalInput").ap()
    out = nc.dram_tensor("out", [3, TPC], F32, kind="ExternalOutput").ap()

    SPP = PIECE // SUPER  # supers per piece (2)

    with tile.TileContext(nc) as tc:
        with ExitStack() as ctx:
            const = ctx.enter_context(tc.tile_pool(name="const", bufs=1))
            lat_pool = ctx.enter_context(tc.tile_pool(name="lat", bufs=16))
            ps_pool = ctx.enter_context(tc.tile_pool(name="ps", bufs=1, space="PSUM"))

            wt_t = const.tile([P, 3 * N_CHUNK], wt_dt)
            nc.scalar.dma_start(wt_t[:], wt[:])
            id3_t = const.tile([3, 3], F16)
            nc.scalar.dma_start(id3_t[:], ident3[:])
            nz_t = const.tile([3, TPC], F16)
            nc.gpsimd.dma_start(nz_t[:], nzt[:])

            for _ in range(repeat):
                pss = [ps_pool.tile([3, SUPER], F32, name=f"ps{s}", tag=f"ps{s}")
                       for s in range(N_SUPER)]
                for k in range(N_CHUNK):
                    w = SUPER if k == 0 else PIECE
                    spp = w // SUPER
                    pieces = []
                    for p in range(TPC // w):
                        lt = lat_pool.tile([P, w], lat_dt, name="lt", tag="lt")
                        nc.sync.dma_start(
                            lt[:], latT[k * P:(k + 1) * P, p * w:(p + 1) * w])
                        pieces.append(lt)
                    for s in range(N_SUPER):
                        nc.tensor.matmul(
                            pss[s][:], wt_t[:, k * 3:(k + 1) * 3],
                            pieces[s // spp][:, (s % spp) * SUPER:
                                             (s % spp + 1) * SUPER],
                            start=(k == 0), stop=False,
                        )
                        if k == N_CHUNK - 1:
                            nc.tensor.matmul(
                                pss[s][:], id3_t[:],
                                nz_t[:, s * SUPER:(s + 1) * SUPER],
                                start=False, stop=True,
                            )
                            eng = nc.sync if s % 2 == 0 else nc.scalar
                            eng.dma_start(
                                out[:, s * SUPER:(s + 1) * SUPER], pss[s][:])

    nc.compile()
    _NC_CACHE[key] = nc
    return nc


def _coeff(T: int) -> float:
    a = 1.0
    for t in range(T):
        a *= (t + 1) / T
    return a


PIPELINE = "v8_fp8"  # "v7" | "v8_fp16" | "v8_fp8" | "v9_fp16" | "v9_fp8"


def kernel(latent, W, b, noise, diffusion_steps, _trace=False, _pipeline=None):
    import ml_dtypes
    T = int(diffusion_steps)
    A = _coeff(T)
    pipeline = _pipeline or PIPELINE
    fp8 = pipeline.endswith("fp8")
    v9 = pipeline.startswith("v9")

    lat_flat = np.ascontiguousarray(latent.reshape(TOK, D), dtype=np.float32)
    if fp8:
        latT_h = lat_flat.astype(ml_dtypes.float8_e3m4).T  # [D, TOK] view
    else:
        latT_h = lat_flat.astype(np.float16).T
    wt_eff = np.ascontiguousarray(W.T).astype(np.float32) * np.float32(1.0 - A)
    # prepack [2048, 3] -> [128, 16*3]: chunk k (rows 128k..128k+128) at cols 3k..3k+3
    wt_packed = np.ascontiguousarray(
        wt_eff.reshape(N_CHUNK, P, 3).transpose(1, 0, 2).reshape(P, 3 * N_CHUNK)
    ).astype(np.float16)
    nz_eff = (np.float32(A) * noise.reshape(TOK, 3)
              + np.float32(1.0 - A) * b[None, :].astype(np.float32))
    nz_dt = np.float16 if v9 else np.float32
    nz_eff_t = np.ascontiguousarray(nz_eff.T.astype(nz_dt))  # [3, TOK]

    lat_dt = mybir.dt.float8e3 if fp8 else F16
    if pipeline == "v7":
        nc = _build_nc_v7()
    elif v9:
        nc = _build_nc_v9(lat_dt=lat_dt)
    else:
        nc = _build_nc_v8(lat_dt=lat_dt)
    in_maps = []
    for c in range(N_CORES):
        im = {
            "latT": np.ascontiguousarray(latT_h[:, c * TPC:(c + 1) * TPC]),
            "wt": wt_packed,
            "nzt": np.ascontiguousarray(nz_eff_t[:, c * TPC:(c + 1) * TPC]),
        }
        if v9:
            im["ident3"] = np.eye(3, dtype=np.float16)
        in_maps.append(im)
    res = run_bass_kernel_spmd(nc, in_maps, core_ids=list(range(N_CORES)),
                               trace=_trace)
    out = np.empty((TOK, 3), dtype=np.float32)
    for c in range(N_CORES):
        out[c * TPC:(c + 1) * TPC] = res.results[c]["out"].T
    if _trace:
        kernel._last_results = res
    return out.reshape(B, S, 3)


# revision 22
# speedup vs baseline: 1.0962x; 1.0685x over previous
"""Trainium2 Bass kernel for DiffusionCoordinateInitializer.

Reference computation:
    coords = einsum("bsd,cd->bsc", latent, W) + b          # [B, S, 3]
    x = noise; for t in reversed(range(T)): x = a*x + (1-a)*coords, a=(t+1)/T
which collapses (affine fixed-point iteration) to
    x = A*noise + (1-A)*(coords + b),  A = prod_{t=1..T} t/T = T!/T^T

Strategy (pure data-parallel over 8 cores, token-sharded):
  - Host folds (1-A) into W^T and A*noise + (1-A)*b into a bias tensor, so
    the device computes out^T[3, tok] = (W_eff @ latent^T) + bias^T.
  - Host pre-transposes + downcasts latent to fp16 [d, tok] per core, so the
    device streams contraction-major tiles straight into accumulating
    matmuls: no on-chip transposes (v5's PE bottleneck) and half the HBM
    traffic (DMA floor ~47 us/core instead of ~94 us).
  - Chunk-major schedule: for each 128-row d-chunk, one [128, 4096] fp16
    load feeds 8 skinny matmuls (W chunk stationary [128, 3], moving
    [128, 512]) accumulating into 8 PSUM banks, one per 512-token super.
  - DVE adds the bias tensor out of PSUM; one [3, 4096] store per core.
"""

import numpy as np
from contextlib import ExitStack

import concourse.bass as bass  # noqa: F401
import concourse.tile as tile
from concourse import bacc, mybir
from concourse.bass_utils import run_bass_kernel_spmd

N_CORES = 8
B, S, D = 4, 8192, 2048
TOK = B * S                      # 32768
TPC = TOK // N_CORES             # 4096 tokens per core
P = 128
SUPER = 512                      # tokens per PSUM bank (max psum free f32)
N_SUPER = TPC // SUPER           # 8
N_CHUNK = D // P                 # 16
F32 = mybir.dt.float32
F16 = mybir.dt.float16
F8E3 = mybir.dt.float8e3

_NC_CACHE = {}


def _build_nc_v7(lat_dt=F16, repeat=1):
    """Pre-transposed stream: latT [D, TPC] (fp16 or fp8e3m4) in DRAM,
    chunk-major accumulating matmuls into 8 PSUM banks, no transposes."""
    key = ("v7", lat_dt, repeat)
    if key in _NC_CACHE:
        return _NC_CACHE[key]

    nc = bacc.Bacc("TRN2", target_bir_lowering=False, debug=False,
                   enable_asserts=False, num_devices=N_CORES)
    latT = nc.dram_tensor("latT", [D, TPC], lat_dt, kind="ExternalInput").ap()
    # host prepacks W_eff^T chunks as [128, 16*3]: wt[p, 3k+c] = W_eff[c, 128k+p]
    wt = nc.dram_tensor("wt", [P, 3 * N_CHUNK], F16, kind="ExternalInput").ap()
    nzt = nc.dram_tensor("nzt", [3, TPC], F32, kind="ExternalInput").ap()
    out = nc.dram_tensor("out", [3, TPC], F32, kind="ExternalOutput").ap()

    with tile.TileContext(nc) as tc:
        with ExitStack() as ctx:
            const = ctx.enter_context(tc.tile_pool(name="const", bufs=1))
            lat_pool = ctx.enter_context(tc.tile_pool(name="lat", bufs=4))
            ps_pool = ctx.enter_context(tc.tile_pool(name="ps", bufs=1, space="PSUM"))
            osb_pool = ctx.enter_context(tc.tile_pool(name="osb", bufs=2))

            wt_t = const.tile([P, 3 * N_CHUNK], F16)
            nc.sync.dma_start(wt_t[:], wt[:])
            nz_t = const.tile([3, TPC], F32)
            nc.sync.dma_start(nz_t[:], nzt[:])

            for _ in range(repeat):
                pss = [ps_pool.tile([3, SUPER], F32, name=f"ps{s}", tag=f"ps{s}")
                       for s in range(N_SUPER)]
                for k in range(N_CHUNK):
                    lt = lat_pool.tile([P, TPC], F16, name="lt", tag="lt")
                    nc.sync.dma_start(lt[:], latT[k * P:(k + 1) * P, :])
                    for s in range(N_SUPER):
                        nc.tensor.matmul(
                            pss[s][:], wt_t[:, k * 3:(k + 1) * 3],
                            lt[:, s * SUPER:(s + 1) * SUPER],
                            start=(k == 0), stop=(k == N_CHUNK - 1),
                        )
                osb = osb_pool.tile([3, TPC], F32, name="osb", tag="osb")
                for s in range(N_SUPER):
                    nc.vector.tensor_add(osb[:, s * SUPER:(s + 1) * SUPER],
                                         pss[s][:], nz_t[:, s * SUPER:(s + 1) * SUPER])
                nc.sync.dma_start(out[:], osb[:])

    nc.compile()
    _NC_CACHE[key] = nc
    return nc


PIECE = 1024                     # tokens per DMA piece (2 KB/part fp16)
N_PIECE = TPC // PIECE           # 4 pieces per chunk


def _build_nc_v8(lat_dt=F16, wt_dt=F16, repeat=1):
    """Piece-granular stream + interleaved drain.

    Same math as v7 but: each 128-row d-chunk is loaded as 4 [128, 1024]
    pieces so the first matmul starts ~8 us earlier; after the last chunk,
    each super's bias-add runs on alternating Vector/Scalar engines right
    behind its stop-matmul, and its [3, 512] store issues immediately --
    the drain hides under the PE tail instead of serializing after it.
    """
    key = ("v8", lat_dt, wt_dt, repeat)
    if key in _NC_CACHE:
        return _NC_CACHE[key]

    nc = bacc.Bacc("TRN2", target_bir_lowering=False, debug=False,
                   enable_asserts=False, num_devices=N_CORES)
    latT = nc.dram_tensor("latT", [D, TPC], lat_dt, kind="ExternalInput").ap()
    wt = nc.dram_tensor("wt", [P, 3 * N_CHUNK], wt_dt, kind="ExternalInput").ap()
    nzt = nc.dram_tensor("nzt", [3, TPC], F32, kind="ExternalInput").ap()
    out = nc.dram_tensor("out", [3, TPC], F32, kind="ExternalOutput").ap()

    SPP = PIECE // SUPER  # supers per piece (2)

    with tile.TileContext(nc) as tc:
        with ExitStack() as ctx:
            const = ctx.enter_context(tc.tile_pool(name="const", bufs=1))
            lat_pool = ctx.enter_context(tc.tile_pool(name="lat", bufs=16))
            ps_pool = ctx.enter_context(tc.tile_pool(name="ps", bufs=1, space="PSUM"))
            osb_pool = ctx.enter_context(tc.tile_pool(name="osb", bufs=8))

            # consts via engine sequencers: the Sync sequencer spends the
            # first ~9 us on queue init, and a DIRECT2D issued there would
            # gate the first matmul on the weights until ~11 us.
            wt_t = const.tile([P, 3 * N_CHUNK], wt_dt)
            nc.scalar.dma_start(wt_t[:], wt[:])
            nz_t = const.tile([3, TPC], F32)
            nc.gpsimd.dma_start(nz_t[:], nzt[:])

            for _ in range(repeat):
                pss = [ps_pool.tile([3, SUPER], F32, name=f"ps{s}", tag=f"ps{s}")
                       for s in range(N_SUPER)]
                for k in range(N_CHUNK):
                    # chunk 0 in 512-token pieces so the first matmul's
                    # dependency lands ~2 us after DMA start; 1024 after
                    w = SUPER if k == 0 else PIECE
                    spp = w // SUPER
                    pieces = []
                    for p in range(TPC // w):
                        lt = lat_pool.tile([P, w], lat_dt, name="lt", tag="lt")
                        nc.sync.dma_start(
                            lt[:], latT[k * P:(k + 1) * P, p * w:(p + 1) * w])
                        pieces.append(lt)
                    for s in range(N_SUPER):
                        nc.tensor.matmul(
                            pss[s][:], wt_t[:, k * 3:(k + 1) * 3],
                            pieces[s // spp][:, (s % spp) * SUPER:
                                             (s % spp + 1) * SUPER],
                            start=(k == 0), stop=(k == N_CHUNK - 1),
                        )
                        if k == N_CHUNK - 1:
                            osb = osb_pool.tile([3, SUPER], F32,
                                                name="osb", tag="osb")
                            nc.vector.tensor_add(osb[:], pss[s][:],
                                                 nz_t[:, s * SUPER:(s + 1) * SUPER])
                            nc.scalar.dma_start(
                                out[:, s * SUPER:(s + 1) * SUPER], osb[:])

    nc.compile()
    _NC_CACHE[key] = nc
    return nc


def _build_nc_v9(lat_dt=F16, wt_dt=F16, repeat=1):
    """v8 + bias-add folded into the PE and stores straight from PSUM.

    The noise/bias term enters each super's accumulation group as one extra
    matmul: stationary = I3 [3, 3], moving = nz16 [3, 512] fp16, so
    psum += I3^T @ nz = nz elementwise. No Vector/Scalar engine work at
    all; each super's [3, 512] result DMAs from PSUM as soon as its group
    stops, hiding the whole drain under the PE tail.
    """
    key = ("v9", lat_dt, wt_dt, repeat)
    if key in _NC_CACHE:
        return _NC_CACHE[key]

    nc = bacc.Bacc("TRN2", target_bir_lowering=False, debug=False,
                   enable_asserts=False, num_devices=N_CORES)
    latT = nc.dram_tensor("latT", [D, TPC], lat_dt, kind="ExternalInput").ap()
    wt = nc.dram_tensor("wt", [P, 3 * N_CHUNK], wt_dt, kind="ExternalInput").ap()
    nzt = nc.dram_tensor("nzt", [3, TPC], F32, kind="ExternalInput").ap()
    out = nc.dram_tensor("out", [3, TPC], F32, kind="ExternalOutput").ap()

    with tile.TileContext(nc) as tc:
        with ExitStack() as ctx:
            const = ctx.enter_context(tc.tile_pool(name="const", bufs=1))
            lat_pool = ctx.enter_context(tc.tile_pool(name="lat", bufs=32))
            ps_pool = ctx.enter_context(tc.tile_pool(name="ps", bufs=1, space="PSUM"))
            osb_pool = ctx.enter_context(tc.tile_pool(name="osb", bufs=4))

            wt_t = const.tile([P, 3 * N_CHUNK], wt_dt)
            nc.scalar.dma_start(wt_t[:], wt[:])
            nz_t = const.tile([3, TPC], F32)
            nc.gpsimd.dma_start(nz_t[:], nzt[:])

            for _ in range(repeat):
                pss = [ps_pool.tile([3, SUPER], F32, name=f"ps{s}", tag=f"ps{s}")
                       for s in range(N_SUPER)]
                # token-pair-major: each 1024-token pair streams all 16
                # chunks, closes its two accumulation groups, and drains
                # while the next pair streams -- no end-of-kernel drain.
                for pr in range(N_SUPER // 2):
                    pieces = []
                    for k in range(N_CHUNK):
                        lt = lat_pool.tile([P, PIECE], lat_dt, name="lt", tag="lt")
                        nc.sync.dma_start(
                            lt[:], latT[k * P:(k + 1) * P,
                                        pr * PIECE:(pr + 1) * PIECE])
                        pieces.append(lt)
                    for k in range(N_CHUNK):
                        for j in range(2):
                            s = 2 * pr + j
                            nc.tensor.matmul(
                                pss[s][:], wt_t[:, k * 3:(k + 1) * 3],
                                pieces[k][:, j * SUPER:(j + 1) * SUPER],
                                start=(k == 0), stop=(k == N_CHUNK - 1),
                            )
                    for j in range(2):
                        s = 2 * pr + j
                        osb = osb_pool.tile([3, SUPER], F32, name="osb", tag="osb")
                        nc.vector.tensor_add(osb[:], pss[s][:],
                                             nz_t[:, s * SUPER:(s + 1) * SUPER])
                        eng = nc.sync if j == 0 else nc.scalar
                        eng.dma_start(
                            out[:, s * SUPER:(s + 1) * SUPER], osb[:])

    nc.compile()
    _NC_CACHE[key] = nc
    return nc


def _coeff(T: int) -> float:
    a = 1.0
    for t in range(T):
        a *= (t + 1) / T
    return a


PIPELINE = "v8_fp8"  # "v7" | "v8_fp16" | "v8_fp8" | "v9_fp16" | "v9_fp8"


def kernel(latent, W, b, noise, diffusion_steps, _trace=False, _pipeline=None):
    import ml_dtypes
    T = int(diffusion_steps)
    A = _coeff(T)
    pipeline = _pipeline or PIPELINE
    fp8 = pipeline.endswith("fp8")
    v9 = pipeline.startswith("v9")

    lat_flat = np.ascontiguousarray(latent.reshape(TOK, D), dtype=np.float32)
    if fp8:
        latT_h = lat_flat.astype(ml_dtypes.float8_e3m4).T  # [D, TOK] view
    else:
        latT_h = lat_flat.astype(np.float16).T
    wt_eff = np.ascontiguousarray(W.T).astype(np.float32) * np.float32(1.0 - A)
    # prepack [2048, 3] -> [128, 16*3]: chunk k (rows 128k..128k+128) at cols 3k..3k+3
    wt_packed = np.ascontiguousarray(
        wt_eff.reshape(N_CHUNK, P, 3).transpose(1, 0, 2).reshape(P, 3 * N_CHUNK)
    ).astype(np.float16)
    nz_eff = (np.float32(A) * noise.reshape(TOK, 3)
              + np.float32(1.0 - A) * b[None, :].astype(np.float32))
    nz_eff_t = np.ascontiguousarray(nz_eff.T.astype(np.float32))  # [3, TOK]

    lat_dt = mybir.dt.float8e3 if fp8 else F16
    if pipeline == "v7":
        nc = _build_nc_v7()
    elif v9:
        nc = _build_nc_v9(lat_dt=lat_dt)
    else:
        nc = _build_nc_v8(lat_dt=lat_dt)
    in_maps = []
    for c in range(N_CORES):
        in_maps.append({
            "latT": np.ascontiguousarray(latT_h[:, c * TPC:(c + 1) * TPC]),
            "wt": wt_packed,
            "nzt": np.ascontiguousarray(nz_eff_t[:, c * TPC:(c + 1) * TPC]),
        })
    res = run_bass_kernel_spmd(nc, in_maps, core_ids=list(range(N_CORES)),
                               trace=_trace)
    out = np.empty((TOK, 3), dtype=np.float32)
    for c in range(N_CORES):
        out[c * TPC:(c + 1) * TPC] = res.results[c]["out"].T
    if _trace:
        kernel._last_results = res
    return out.reshape(B, S, 3)


# revision 25
# speedup vs baseline: 1.2858x; 1.1730x over previous
"""Trainium2 Bass kernel for DiffusionCoordinateInitializer.

Reference computation:
    coords = einsum("bsd,cd->bsc", latent, W) + b          # [B, S, 3]
    x = noise; for t in reversed(range(T)): x = a*x + (1-a)*coords, a=(t+1)/T
which collapses (affine fixed-point iteration) to
    x = A*noise + (1-A)*(coords + b),  A = prod_{t=1..T} t/T = T!/T^T

Strategy (pure data-parallel over 8 cores, token-sharded):
  - Host folds (1-A) into W^T and A*noise + (1-A)*b into a bias tensor, so
    the device computes out^T[3, tok] = (W_eff @ latent^T) + bias^T.
  - Host pre-transposes + downcasts latent to fp16 [d, tok] per core, so the
    device streams contraction-major tiles straight into accumulating
    matmuls: no on-chip transposes (v5's PE bottleneck) and half the HBM
    traffic (DMA floor ~47 us/core instead of ~94 us).
  - Chunk-major schedule: for each 128-row d-chunk, one [128, 4096] fp16
    load feeds 8 skinny matmuls (W chunk stationary [128, 3], moving
    [128, 512]) accumulating into 8 PSUM banks, one per 512-token super.
  - DVE adds the bias tensor out of PSUM; one [3, 4096] store per core.
"""

import numpy as np
from contextlib import ExitStack

import concourse.bass as bass  # noqa: F401
import concourse.tile as tile
from concourse import bacc, mybir
from concourse.bass_utils import run_bass_kernel_spmd

N_CORES = 8
B, S, D = 4, 8192, 2048
TOK = B * S                      # 32768
TPC = TOK // N_CORES             # 4096 tokens per core
P = 128
SUPER = 512                      # tokens per PSUM bank (max psum free f32)
N_SUPER = TPC // SUPER           # 8
N_CHUNK = D // P                 # 16
F32 = mybir.dt.float32
F16 = mybir.dt.float16
F8E3 = mybir.dt.float8e3

_NC_CACHE = {}


def _build_nc_v7(lat_dt=F16, repeat=1):
    """Pre-transposed stream: latT [D, TPC] (fp16 or fp8e3m4) in DRAM,
    chunk-major accumulating matmuls into 8 PSUM banks, no transposes."""
    key = ("v7", lat_dt, repeat)
    if key in _NC_CACHE:
        return _NC_CACHE[key]

    nc = bacc.Bacc("TRN2", target_bir_lowering=False, debug=False,
                   enable_asserts=False, num_devices=N_CORES)
    latT = nc.dram_tensor("latT", [D, TPC], lat_dt, kind="ExternalInput").ap()
    # host prepacks W_eff^T chunks as [128, 16*3]: wt[p, 3k+c] = W_eff[c, 128k+p]
    wt = nc.dram_tensor("wt", [P, 3 * N_CHUNK], F16, kind="ExternalInput").ap()
    nzt = nc.dram_tensor("nzt", [3, TPC], F32, kind="ExternalInput").ap()
    out = nc.dram_tensor("out", [3, TPC], F32, kind="ExternalOutput").ap()

    with tile.TileContext(nc) as tc:
        with ExitStack() as ctx:
            const = ctx.enter_context(tc.tile_pool(name="const", bufs=1))
            lat_pool = ctx.enter_context(tc.tile_pool(name="lat", bufs=4))
            ps_pool = ctx.enter_context(tc.tile_pool(name="ps", bufs=1, space="PSUM"))
            osb_pool = ctx.enter_context(tc.tile_pool(name="osb", bufs=2))

            wt_t = const.tile([P, 3 * N_CHUNK], F16)
            nc.sync.dma_start(wt_t[:], wt[:])
            nz_t = const.tile([3, TPC], F32)
            nc.sync.dma_start(nz_t[:], nzt[:])

            for _ in range(repeat):
                pss = [ps_pool.tile([3, SUPER], F32, name=f"ps{s}", tag=f"ps{s}")
                       for s in range(N_SUPER)]
                for k in range(N_CHUNK):
                    lt = lat_pool.tile([P, TPC], F16, name="lt", tag="lt")
                    nc.sync.dma_start(lt[:], latT[k * P:(k + 1) * P, :])
                    for s in range(N_SUPER):
                        nc.tensor.matmul(
                            pss[s][:], wt_t[:, k * 3:(k + 1) * 3],
                            lt[:, s * SUPER:(s + 1) * SUPER],
                            start=(k == 0), stop=(k == N_CHUNK - 1),
                        )
                osb = osb_pool.tile([3, TPC], F32, name="osb", tag="osb")
                for s in range(N_SUPER):
                    nc.vector.tensor_add(osb[:, s * SUPER:(s + 1) * SUPER],
                                         pss[s][:], nz_t[:, s * SUPER:(s + 1) * SUPER])
                nc.sync.dma_start(out[:], osb[:])

    nc.compile()
    _NC_CACHE[key] = nc
    return nc


PIECE = 1024                     # tokens per DMA piece (2 KB/part fp16)
N_PIECE = TPC // PIECE           # 4 pieces per chunk


def _build_nc_v8(lat_dt=F16, wt_dt=F16, repeat=1):
    """Piece-granular stream + interleaved drain.

    Same math as v7 but: each 128-row d-chunk is loaded as 4 [128, 1024]
    pieces so the first matmul starts ~8 us earlier; after the last chunk,
    each super's bias-add runs on alternating Vector/Scalar engines right
    behind its stop-matmul, and its [3, 512] store issues immediately --
    the drain hides under the PE tail instead of serializing after it.
    """
    key = ("v8", lat_dt, wt_dt, repeat)
    if key in _NC_CACHE:
        return _NC_CACHE[key]

    nc = bacc.Bacc("TRN2", target_bir_lowering=False, debug=False,
                   enable_asserts=False, num_devices=N_CORES)
    latT = nc.dram_tensor("latT", [D, TPC], lat_dt, kind="ExternalInput").ap()
    wt = nc.dram_tensor("wt", [P, 3 * N_CHUNK], wt_dt, kind="ExternalInput").ap()
    nzt = nc.dram_tensor("nzt", [3, TPC], F32, kind="ExternalInput").ap()
    out = nc.dram_tensor("out", [3, TPC], F32, kind="ExternalOutput").ap()

    SPP = PIECE // SUPER  # supers per piece (2)

    with tile.TileContext(nc) as tc:
        with ExitStack() as ctx:
            const = ctx.enter_context(tc.tile_pool(name="const", bufs=1))
            lat_pool = ctx.enter_context(tc.tile_pool(name="lat", bufs=16))
            ps_pool = ctx.enter_context(tc.tile_pool(name="ps", bufs=1, space="PSUM"))
            osb_pool = ctx.enter_context(tc.tile_pool(name="osb", bufs=8))

            # consts via engine sequencers: the Sync sequencer spends the
            # first ~9 us on queue init, and a DIRECT2D issued there would
            # gate the first matmul on the weights until ~11 us.
            wt_t = const.tile([P, 3 * N_CHUNK], wt_dt)
            nc.scalar.dma_start(wt_t[:], wt[:])
            nz_t = const.tile([3, TPC], F32)
            nc.gpsimd.dma_start(nz_t[:], nzt[:])

            for _ in range(repeat):
                pss = [ps_pool.tile([3, SUPER], F32, name=f"ps{s}", tag=f"ps{s}")
                       for s in range(N_SUPER)]
                for k in range(N_CHUNK):
                    # chunk 0 in 512-token pieces so the first matmul's
                    # dependency lands ~2 us after DMA start; 1024 after
                    w = SUPER if k == 0 else PIECE
                    spp = w // SUPER
                    pieces = []
                    for p in range(TPC // w):
                        lt = lat_pool.tile([P, w], lat_dt, name="lt", tag="lt")
                        nc.sync.dma_start(
                            lt[:], latT[k * P:(k + 1) * P, p * w:(p + 1) * w])
                        pieces.append(lt)
                    for s in range(N_SUPER):
                        nc.tensor.matmul(
                            pss[s][:], wt_t[:, k * 3:(k + 1) * 3],
                            pieces[s // spp][:, (s % spp) * SUPER:
                                             (s % spp + 1) * SUPER],
                            start=(k == 0), stop=(k == N_CHUNK - 1),
                        )
                        if k == N_CHUNK - 1:
                            osb = osb_pool.tile([3, SUPER], F32,
                                                name="osb", tag="osb")
                            nc.vector.tensor_add(osb[:], pss[s][:],
                                                 nz_t[:, s * SUPER:(s + 1) * SUPER])
                            nc.scalar.dma_start(
                                out[:, s * SUPER:(s + 1) * SUPER], osb[:])

    nc.compile()
    _NC_CACHE[key] = nc
    return nc


def _build_nc_v10(lat_dt=F16, wt_dt=F16, group=2, bufs=6, repeat=1):
    """Pair-major with multi-chunk DMA pieces.

    latT3 [128, 16, TPC] host layout (partition-major) lets one DMA carry
    `group` chunks for a 1024-token pair: [128, group, 1024] -> SBUF
    [128, group*1024]. Fewer, bigger transfers = fewer PE semaphore waits
    (the ~0.2 us/piece stall tax v9 measured with 64 pieces).
    """
    key = ("v10", lat_dt, wt_dt, group, bufs, repeat)
    if key in _NC_CACHE:
        return _NC_CACHE[key]

    nc = bacc.Bacc("TRN2", target_bir_lowering=False, debug=False,
                   enable_asserts=False, num_devices=N_CORES)
    latT3 = nc.dram_tensor("latT", [P, N_CHUNK, TPC], lat_dt,
                           kind="ExternalInput").ap()
    wt = nc.dram_tensor("wt", [P, 3 * N_CHUNK], wt_dt, kind="ExternalInput").ap()
    nzt = nc.dram_tensor("nzt", [3, TPC], F32, kind="ExternalInput").ap()
    out = nc.dram_tensor("out", [3, TPC], F32, kind="ExternalOutput").ap()

    NG = N_CHUNK // group

    with tile.TileContext(nc) as tc:
        with ExitStack() as ctx:
            const = ctx.enter_context(tc.tile_pool(name="const", bufs=1))
            lat_pool = ctx.enter_context(tc.tile_pool(name="lat", bufs=bufs))
            ps_pool = ctx.enter_context(tc.tile_pool(name="ps", bufs=1, space="PSUM"))
            osb_pool = ctx.enter_context(tc.tile_pool(name="osb", bufs=4))

            wt_t = const.tile([P, 3 * N_CHUNK], wt_dt)
            nc.scalar.dma_start(wt_t[:], wt[:])
            nz_t = const.tile([3, TPC], F32)
            nc.gpsimd.dma_start(nz_t[:], nzt[:])

            for _ in range(repeat):
                pss = [ps_pool.tile([3, SUPER], F32, name=f"ps{s}", tag=f"ps{s}")
                       for s in range(N_SUPER)]
                for pr in range(N_SUPER // 2):
                    pieces = []
                    for g in range(NG):
                        lt = lat_pool.tile([P, group * PIECE], lat_dt,
                                           name="lt", tag="lt")
                        nc.sync.dma_start(
                            lt[:], latT3[:, g * group:(g + 1) * group,
                                         pr * PIECE:(pr + 1) * PIECE])
                        pieces.append(lt)
                    for k in range(N_CHUNK):
                        g, i = divmod(k, group)
                        for j in range(2):
                            s = 2 * pr + j
                            nc.tensor.matmul(
                                pss[s][:], wt_t[:, k * 3:(k + 1) * 3],
                                pieces[g][:, i * PIECE + j * SUPER:
                                         i * PIECE + (j + 1) * SUPER],
                                start=(k == 0), stop=(k == N_CHUNK - 1),
                            )
                    for j in range(2):
                        s = 2 * pr + j
                        osb = osb_pool.tile([3, SUPER], F32, name="osb", tag="osb")
                        nc.vector.tensor_add(osb[:], pss[s][:],
                                             nz_t[:, s * SUPER:(s + 1) * SUPER])
                        eng = nc.sync if j == 0 else nc.scalar
                        eng.dma_start(
                            out[:, s * SUPER:(s + 1) * SUPER], osb[:])

    nc.compile()
    _NC_CACHE[key] = nc
    return nc


def _build_nc_v9(lat_dt=F16, wt_dt=F16, repeat=1):
    """v8 + bias-add folded into the PE and stores straight from PSUM.

    The noise/bias term enters each super's accumulation group as one extra
    matmul: stationary = I3 [3, 3], moving = nz16 [3, 512] fp16, so
    psum += I3^T @ nz = nz elementwise. No Vector/Scalar engine work at
    all; each super's [3, 512] result DMAs from PSUM as soon as its group
    stops, hiding the whole drain under the PE tail.
    """
    key = ("v9", lat_dt, wt_dt, repeat)
    if key in _NC_CACHE:
        return _NC_CACHE[key]

    nc = bacc.Bacc("TRN2", target_bir_lowering=False, debug=False,
                   enable_asserts=False, num_devices=N_CORES)
    latT = nc.dram_tensor("latT", [D, TPC], lat_dt, kind="ExternalInput").ap()
    wt = nc.dram_tensor("wt", [P, 3 * N_CHUNK], wt_dt, kind="ExternalInput").ap()
    nzt = nc.dram_tensor("nzt", [3, TPC], F32, kind="ExternalInput").ap()
    out = nc.dram_tensor("out", [3, TPC], F32, kind="ExternalOutput").ap()

    with tile.TileContext(nc) as tc:
        with ExitStack() as ctx:
            const = ctx.enter_context(tc.tile_pool(name="const", bufs=1))
            lat_pool = ctx.enter_context(tc.tile_pool(name="lat", bufs=32))
            ps_pool = ctx.enter_context(tc.tile_pool(name="ps", bufs=1, space="PSUM"))
            osb_pool = ctx.enter_context(tc.tile_pool(name="osb", bufs=4))

            wt_t = const.tile([P, 3 * N_CHUNK], wt_dt)
            nc.scalar.dma_start(wt_t[:], wt[:])
            nz_t = const.tile([3, TPC], F32)
            nc.gpsimd.dma_start(nz_t[:], nzt[:])

            for _ in range(repeat):
                pss = [ps_pool.tile([3, SUPER], F32, name=f"ps{s}", tag=f"ps{s}")
                       for s in range(N_SUPER)]
                # token-pair-major: each 1024-token pair streams all 16
                # chunks, closes its two accumulation groups, and drains
                # while the next pair streams -- no end-of-kernel drain.
                for pr in range(N_SUPER // 2):
                    pieces = []
                    for k in range(N_CHUNK):
                        lt = lat_pool.tile([P, PIECE], lat_dt, name="lt", tag="lt")
                        nc.sync.dma_start(
                            lt[:], latT[k * P:(k + 1) * P,
                                        pr * PIECE:(pr + 1) * PIECE])
                        pieces.append(lt)
                    for k in range(N_CHUNK):
                        for j in range(2):
                            s = 2 * pr + j
                            nc.tensor.matmul(
                                pss[s][:], wt_t[:, k * 3:(k + 1) * 3],
                                pieces[k][:, j * SUPER:(j + 1) * SUPER],
                                start=(k == 0), stop=(k == N_CHUNK - 1),
                            )
                    for j in range(2):
                        s = 2 * pr + j
                        osb = osb_pool.tile([3, SUPER], F32, name="osb", tag="osb")
                        nc.vector.tensor_add(osb[:], pss[s][:],
                                             nz_t[:, s * SUPER:(s + 1) * SUPER])
                        eng = nc.sync if j == 0 else nc.scalar
                        eng.dma_start(
                            out[:, s * SUPER:(s + 1) * SUPER], osb[:])

    nc.compile()
    _NC_CACHE[key] = nc
    return nc


def _coeff(T: int) -> float:
    a = 1.0
    for t in range(T):
        a *= (t + 1) / T
    return a


PIPELINE = "v9_fp8"  # "v7" | "v8_*" | "v9_*" | "v10_*" (suffix fp16|fp8)
_V10_GROUP = 2
_V10_BUFS = 6


def kernel(latent, W, b, noise, diffusion_steps, _trace=False, _pipeline=None):
    import ml_dtypes
    T = int(diffusion_steps)
    A = _coeff(T)
    pipeline = _pipeline or PIPELINE
    fp8 = pipeline.endswith("fp8")
    v9 = pipeline.startswith("v9")

    lat_flat = np.ascontiguousarray(latent.reshape(TOK, D), dtype=np.float32)
    if fp8:
        latT_h = lat_flat.astype(ml_dtypes.float8_e3m4).T  # [D, TOK] view
    else:
        latT_h = lat_flat.astype(np.float16).T
    wt_eff = np.ascontiguousarray(W.T).astype(np.float32) * np.float32(1.0 - A)
    # prepack [2048, 3] -> [128, 16*3]: chunk k (rows 128k..128k+128) at cols 3k..3k+3
    wt_packed = np.ascontiguousarray(
        wt_eff.reshape(N_CHUNK, P, 3).transpose(1, 0, 2).reshape(P, 3 * N_CHUNK)
    ).astype(np.float16)
    nz_eff = (np.float32(A) * noise.reshape(TOK, 3)
              + np.float32(1.0 - A) * b[None, :].astype(np.float32))
    nz_eff_t = np.ascontiguousarray(nz_eff.T.astype(np.float32))  # [3, TOK]

    lat_dt = mybir.dt.float8e3 if fp8 else F16
    v10 = pipeline.startswith("v10")
    if pipeline == "v7":
        nc = _build_nc_v7()
    elif v10:
        nc = _build_nc_v10(lat_dt=lat_dt, group=_V10_GROUP, bufs=_V10_BUFS)
    elif v9:
        nc = _build_nc_v9(lat_dt=lat_dt)
    else:
        nc = _build_nc_v8(lat_dt=lat_dt)
    if v10:
        # [D, TOK] -> [128, 16, TOK]: partition-major chunk layout
        lat_p = np.ascontiguousarray(
            latT_h.reshape(N_CHUNK, P, TOK).transpose(1, 0, 2))
    in_maps = []
    for c in range(N_CORES):
        in_maps.append({
            "latT": (np.ascontiguousarray(lat_p[:, :, c * TPC:(c + 1) * TPC])
                     if v10 else
                     np.ascontiguousarray(latT_h[:, c * TPC:(c + 1) * TPC])),
            "wt": wt_packed,
            "nzt": np.ascontiguousarray(nz_eff_t[:, c * TPC:(c + 1) * TPC]),
        })
    res = run_bass_kernel_spmd(nc, in_maps, core_ids=list(range(N_CORES)),
                               trace=_trace)
    out = np.empty((TOK, 3), dtype=np.float32)
    for c in range(N_CORES):
        out[c * TPC:(c + 1) * TPC] = res.results[c]["out"].T
    if _trace:
        kernel._last_results = res
    return out.reshape(B, S, 3)


# revision 26
# speedup vs baseline: 1.4272x; 1.1100x over previous
"""Trainium2 Bass kernel for DiffusionCoordinateInitializer.

Reference computation:
    coords = einsum("bsd,cd->bsc", latent, W) + b          # [B, S, 3]
    x = noise; for t in reversed(range(T)): x = a*x + (1-a)*coords, a=(t+1)/T
which collapses (affine fixed-point iteration) to
    x = A*noise + (1-A)*(coords + b),  A = prod_{t=1..T} t/T = T!/T^T

Strategy (pure data-parallel over 8 cores, token-sharded):
  - Host folds (1-A) into W^T and A*noise + (1-A)*b into a bias tensor, so
    the device computes out^T[3, tok] = (W_eff @ latent^T) + bias^T.
  - Host pre-transposes + downcasts latent to fp16 [d, tok] per core, so the
    device streams contraction-major tiles straight into accumulating
    matmuls: no on-chip transposes (v5's PE bottleneck) and half the HBM
    traffic (DMA floor ~47 us/core instead of ~94 us).
  - Chunk-major schedule: for each 128-row d-chunk, one [128, 4096] fp16
    load feeds 8 skinny matmuls (W chunk stationary [128, 3], moving
    [128, 512]) accumulating into 8 PSUM banks, one per 512-token super.
  - DVE adds the bias tensor out of PSUM; one [3, 4096] store per core.
"""

import numpy as np
from contextlib import ExitStack

import concourse.bass as bass  # noqa: F401
import concourse.tile as tile
from concourse import bacc, mybir
from concourse.bass_utils import run_bass_kernel_spmd

N_CORES = 8
B, S, D = 4, 8192, 2048
TOK = B * S                      # 32768
TPC = TOK // N_CORES             # 4096 tokens per core
P = 128
SUPER = 512                      # tokens per PSUM bank (max psum free f32)
N_SUPER = TPC // SUPER           # 8
N_CHUNK = D // P                 # 16
F32 = mybir.dt.float32
F16 = mybir.dt.float16
F8E3 = mybir.dt.float8e3

_NC_CACHE = {}


def _build_nc_v7(lat_dt=F16, repeat=1):
    """Pre-transposed stream: latT [D, TPC] (fp16 or fp8e3m4) in DRAM,
    chunk-major accumulating matmuls into 8 PSUM banks, no transposes."""
    key = ("v7", lat_dt, repeat)
    if key in _NC_CACHE:
        return _NC_CACHE[key]

    nc = bacc.Bacc("TRN2", target_bir_lowering=False, debug=False,
                   enable_asserts=False, num_devices=N_CORES)
    latT = nc.dram_tensor("latT", [D, TPC], lat_dt, kind="ExternalInput").ap()
    # host prepacks W_eff^T chunks as [128, 16*3]: wt[p, 3k+c] = W_eff[c, 128k+p]
    wt = nc.dram_tensor("wt", [P, 3 * N_CHUNK], F16, kind="ExternalInput").ap()
    nzt = nc.dram_tensor("nzt", [3, TPC], F32, kind="ExternalInput").ap()
    out = nc.dram_tensor("out", [3, TPC], F32, kind="ExternalOutput").ap()

    with tile.TileContext(nc) as tc:
        with ExitStack() as ctx:
            const = ctx.enter_context(tc.tile_pool(name="const", bufs=1))
            lat_pool = ctx.enter_context(tc.tile_pool(name="lat", bufs=4))
            ps_pool = ctx.enter_context(tc.tile_pool(name="ps", bufs=1, space="PSUM"))
            osb_pool = ctx.enter_context(tc.tile_pool(name="osb", bufs=2))

            wt_t = const.tile([P, 3 * N_CHUNK], F16)
            nc.sync.dma_start(wt_t[:], wt[:])
            nz_t = const.tile([3, TPC], F32)
            nc.sync.dma_start(nz_t[:], nzt[:])

            for _ in range(repeat):
                pss = [ps_pool.tile([3, SUPER], F32, name=f"ps{s}", tag=f"ps{s}")
                       for s in range(N_SUPER)]
                for k in range(N_CHUNK):
                    lt = lat_pool.tile([P, TPC], F16, name="lt", tag="lt")
                    nc.sync.dma_start(lt[:], latT[k * P:(k + 1) * P, :])
                    for s in range(N_SUPER):
                        nc.tensor.matmul(
                            pss[s][:], wt_t[:, k * 3:(k + 1) * 3],
                            lt[:, s * SUPER:(s + 1) * SUPER],
                            start=(k == 0), stop=(k == N_CHUNK - 1),
                        )
                osb = osb_pool.tile([3, TPC], F32, name="osb", tag="osb")
                for s in range(N_SUPER):
                    nc.vector.tensor_add(osb[:, s * SUPER:(s + 1) * SUPER],
                                         pss[s][:], nz_t[:, s * SUPER:(s + 1) * SUPER])
                nc.sync.dma_start(out[:], osb[:])

    nc.compile()
    _NC_CACHE[key] = nc
    return nc


PIECE = 1024                     # tokens per DMA piece (2 KB/part fp16)
N_PIECE = TPC // PIECE           # 4 pieces per chunk


def _build_nc_v8(lat_dt=F16, wt_dt=F16, repeat=1):
    """Piece-granular stream + interleaved drain.

    Same math as v7 but: each 128-row d-chunk is loaded as 4 [128, 1024]
    pieces so the first matmul starts ~8 us earlier; after the last chunk,
    each super's bias-add runs on alternating Vector/Scalar engines right
    behind its stop-matmul, and its [3, 512] store issues immediately --
    the drain hides under the PE tail instead of serializing after it.
    """
    key = ("v8", lat_dt, wt_dt, repeat)
    if key in _NC_CACHE:
        return _NC_CACHE[key]

    nc = bacc.Bacc("TRN2", target_bir_lowering=False, debug=False,
                   enable_asserts=False, num_devices=N_CORES)
    latT = nc.dram_tensor("latT", [D, TPC], lat_dt, kind="ExternalInput").ap()
    wt = nc.dram_tensor("wt", [P, 3 * N_CHUNK], wt_dt, kind="ExternalInput").ap()
    nzt = nc.dram_tensor("nzt", [3, TPC], F32, kind="ExternalInput").ap()
    out = nc.dram_tensor("out", [3, TPC], F32, kind="ExternalOutput").ap()

    SPP = PIECE // SUPER  # supers per piece (2)

    with tile.TileContext(nc) as tc:
        with ExitStack() as ctx:
            const = ctx.enter_context(tc.tile_pool(name="const", bufs=1))
            lat_pool = ctx.enter_context(tc.tile_pool(name="lat", bufs=16))
            ps_pool = ctx.enter_context(tc.tile_pool(name="ps", bufs=1, space="PSUM"))
            osb_pool = ctx.enter_context(tc.tile_pool(name="osb", bufs=8))

            # consts via engine sequencers: the Sync sequencer spends the
            # first ~9 us on queue init, and a DIRECT2D issued there would
            # gate the first matmul on the weights until ~11 us.
            wt_t = const.tile([P, 3 * N_CHUNK], wt_dt)
            nc.scalar.dma_start(wt_t[:], wt[:])
            nz_t = const.tile([3, TPC], F32)
            nc.gpsimd.dma_start(nz_t[:], nzt[:])

            for _ in range(repeat):
                pss = [ps_pool.tile([3, SUPER], F32, name=f"ps{s}", tag=f"ps{s}")
                       for s in range(N_SUPER)]
                for k in range(N_CHUNK):
                    # chunk 0 in 512-token pieces so the first matmul's
                    # dependency lands ~2 us after DMA start; 1024 after
                    w = SUPER if k == 0 else PIECE
                    spp = w // SUPER
                    pieces = []
                    for p in range(TPC // w):
                        lt = lat_pool.tile([P, w], lat_dt, name="lt", tag="lt")
                        nc.sync.dma_start(
                            lt[:], latT[k * P:(k + 1) * P, p * w:(p + 1) * w])
                        pieces.append(lt)
                    for s in range(N_SUPER):
                        nc.tensor.matmul(
                            pss[s][:], wt_t[:, k * 3:(k + 1) * 3],
                            pieces[s // spp][:, (s % spp) * SUPER:
                                             (s % spp + 1) * SUPER],
                            start=(k == 0), stop=(k == N_CHUNK - 1),
                        )
                        if k == N_CHUNK - 1:
                            osb = osb_pool.tile([3, SUPER], F32,
                                                name="osb", tag="osb")
                            nc.vector.tensor_add(osb[:], pss[s][:],
                                                 nz_t[:, s * SUPER:(s + 1) * SUPER])
                            nc.scalar.dma_start(
                                out[:, s * SUPER:(s + 1) * SUPER], osb[:])

    nc.compile()
    _NC_CACHE[key] = nc
    return nc


def _build_nc_v10(lat_dt=F16, wt_dt=F16, group=2, bufs=6, repeat=1):
    """Pair-major with multi-chunk DMA pieces.

    latT3 [128, 16, TPC] host layout (partition-major) lets one DMA carry
    `group` chunks for a 1024-token pair: [128, group, 1024] -> SBUF
    [128, group*1024]. Fewer, bigger transfers = fewer PE semaphore waits
    (the ~0.2 us/piece stall tax v9 measured with 64 pieces).
    """
    key = ("v10", lat_dt, wt_dt, group, bufs, repeat)
    if key in _NC_CACHE:
        return _NC_CACHE[key]

    nc = bacc.Bacc("TRN2", target_bir_lowering=False, debug=False,
                   enable_asserts=False, num_devices=N_CORES)
    latT3 = nc.dram_tensor("latT", [P, N_CHUNK, TPC], lat_dt,
                           kind="ExternalInput").ap()
    wt = nc.dram_tensor("wt", [P, 3 * N_CHUNK], wt_dt, kind="ExternalInput").ap()
    nzt = nc.dram_tensor("nzt", [3, TPC], F32, kind="ExternalInput").ap()
    out = nc.dram_tensor("out", [3, TPC], F32, kind="ExternalOutput").ap()

    NG = N_CHUNK // group

    with tile.TileContext(nc) as tc:
        with ExitStack() as ctx:
            const = ctx.enter_context(tc.tile_pool(name="const", bufs=1))
            lat_pool = ctx.enter_context(tc.tile_pool(name="lat", bufs=bufs))
            ps_pool = ctx.enter_context(tc.tile_pool(name="ps", bufs=1, space="PSUM"))
            osb_pool = ctx.enter_context(tc.tile_pool(name="osb", bufs=4))

            wt_t = const.tile([P, 3 * N_CHUNK], wt_dt)
            nc.scalar.dma_start(wt_t[:], wt[:])
            nz_t = const.tile([3, TPC], F32)
            nc.gpsimd.dma_start(nz_t[:], nzt[:])

            for _ in range(repeat):
                pss = [ps_pool.tile([3, SUPER], F32, name=f"ps{s}", tag=f"ps{s}")
                       for s in range(N_SUPER)]
                for pr in range(N_SUPER // 2):
                    pieces = []
                    for g in range(NG):
                        lt = lat_pool.tile([P, group * PIECE], lat_dt,
                                           name="lt", tag="lt")
                        nc.sync.dma_start(
                            lt[:], latT3[:, g * group:(g + 1) * group,
                                         pr * PIECE:(pr + 1) * PIECE])
                        pieces.append(lt)
                    for k in range(N_CHUNK):
                        g, i = divmod(k, group)
                        for j in range(2):
                            s = 2 * pr + j
                            nc.tensor.matmul(
                                pss[s][:], wt_t[:, k * 3:(k + 1) * 3],
                                pieces[g][:, i * PIECE + j * SUPER:
                                         i * PIECE + (j + 1) * SUPER],
                                start=(k == 0), stop=(k == N_CHUNK - 1),
                            )
                    for j in range(2):
                        s = 2 * pr + j
                        osb = osb_pool.tile([3, SUPER], F32, name="osb", tag="osb")
                        nc.vector.tensor_add(osb[:], pss[s][:],
                                             nz_t[:, s * SUPER:(s + 1) * SUPER])
                        eng = nc.sync if j == 0 else nc.scalar
                        eng.dma_start(
                            out[:, s * SUPER:(s + 1) * SUPER], osb[:])

    nc.compile()
    _NC_CACHE[key] = nc
    return nc


def _build_nc_v9(lat_dt=F16, wt_dt=F16, repeat=1):
    """v8 + bias-add folded into the PE and stores straight from PSUM.

    The noise/bias term enters each super's accumulation group as one extra
    matmul: stationary = I3 [3, 3], moving = nz16 [3, 512] fp16, so
    psum += I3^T @ nz = nz elementwise. No Vector/Scalar engine work at
    all; each super's [3, 512] result DMAs from PSUM as soon as its group
    stops, hiding the whole drain under the PE tail.
    """
    key = ("v9", lat_dt, wt_dt, repeat)
    if key in _NC_CACHE:
        return _NC_CACHE[key]

    nc = bacc.Bacc("TRN2", target_bir_lowering=False, debug=False,
                   enable_asserts=False, num_devices=N_CORES)
    latT = nc.dram_tensor("latT", [D, TPC], lat_dt, kind="ExternalInput").ap()
    wt = nc.dram_tensor("wt", [P, 3 * N_CHUNK], wt_dt, kind="ExternalInput").ap()
    nzt = nc.dram_tensor("nzt", [3, TPC], F32, kind="ExternalInput").ap()
    out = nc.dram_tensor("out", [3, TPC], F32, kind="ExternalOutput").ap()

    with tile.TileContext(nc) as tc:
        with ExitStack() as ctx:
            const = ctx.enter_context(tc.tile_pool(name="const", bufs=1))
            lat_pool = ctx.enter_context(tc.tile_pool(name="lat", bufs=32))
            ps_pool = ctx.enter_context(tc.tile_pool(name="ps", bufs=1, space="PSUM"))
            osb_pool = ctx.enter_context(tc.tile_pool(name="osb", bufs=4))

            wt_t = const.tile([P, 3 * N_CHUNK], wt_dt)
            nc.scalar.dma_start(wt_t[:], wt[:])
            nz_t = const.tile([3, TPC], F32)
            nc.gpsimd.dma_start(nz_t[:], nzt[:])

            for _ in range(repeat):
                pss = [ps_pool.tile([3, SUPER], F32, name=f"ps{s}", tag=f"ps{s}")
                       for s in range(N_SUPER)]
                # token-pair-major: each 1024-token pair streams all 16
                # chunks, closes its two accumulation groups, and drains
                # while the next pair streams -- no end-of-kernel drain.
                for pr in range(N_SUPER // 2):
                    pieces = []
                    for k in range(N_CHUNK):
                        lt = lat_pool.tile([P, PIECE], lat_dt, name="lt", tag="lt")
                        nc.sync.dma_start(
                            lt[:], latT[k * P:(k + 1) * P,
                                        pr * PIECE:(pr + 1) * PIECE])
                        pieces.append(lt)
                    for k in range(N_CHUNK):
                        for j in range(2):
                            s = 2 * pr + j
                            nc.tensor.matmul(
                                pss[s][:], wt_t[:, k * 3:(k + 1) * 3],
                                pieces[k][:, j * SUPER:(j + 1) * SUPER],
                                start=(k == 0), stop=(k == N_CHUNK - 1),
                            )
                    for j in range(2):
                        s = 2 * pr + j
                        osb = osb_pool.tile([3, SUPER], F32, name="osb", tag="osb")
                        nc.vector.tensor_add(osb[:], pss[s][:],
                                             nz_t[:, s * SUPER:(s + 1) * SUPER])
                        eng = nc.sync if j == 0 else nc.scalar
                        eng.dma_start(
                            out[:, s * SUPER:(s + 1) * SUPER], osb[:])

    nc.compile()
    _NC_CACHE[key] = nc
    return nc


def _coeff(T: int) -> float:
    a = 1.0
    for t in range(T):
        a *= (t + 1) / T
    return a


PIPELINE = "v9_fp8"  # "v7" | "v8_*" | "v9_*" | "v10_*" (suffix fp16|fp8)
_V10_GROUP = 2
_V10_BUFS = 8


def kernel(latent, W, b, noise, diffusion_steps, _trace=False, _pipeline=None):
    import ml_dtypes
    T = int(diffusion_steps)
    A = _coeff(T)
    pipeline = _pipeline or PIPELINE
    fp8 = pipeline.endswith("fp8")
    v9 = pipeline.startswith("v9")

    lat_flat = np.ascontiguousarray(latent.reshape(TOK, D), dtype=np.float32)
    if fp8:
        latT_h = lat_flat.astype(ml_dtypes.float8_e3m4).T  # [D, TOK] view
    else:
        latT_h = lat_flat.astype(np.float16).T
    wt_eff = np.ascontiguousarray(W.T).astype(np.float32) * np.float32(1.0 - A)
    # prepack [2048, 3] -> [128, 16*3]: chunk k (rows 128k..128k+128) at cols 3k..3k+3
    wt_packed = np.ascontiguousarray(
        wt_eff.reshape(N_CHUNK, P, 3).transpose(1, 0, 2).reshape(P, 3 * N_CHUNK)
    ).astype(np.float16)
    nz_eff = (np.float32(A) * noise.reshape(TOK, 3)
              + np.float32(1.0 - A) * b[None, :].astype(np.float32))
    nz_eff_t = np.ascontiguousarray(nz_eff.T.astype(np.float32))  # [3, TOK]

    lat_dt = mybir.dt.float8e3 if fp8 else F16
    v10 = pipeline.startswith("v10")
    if pipeline == "v7":
        nc = _build_nc_v7()
    elif v10:
        nc = _build_nc_v10(lat_dt=lat_dt, group=_V10_GROUP, bufs=_V10_BUFS)
    elif v9:
        nc = _build_nc_v9(lat_dt=lat_dt)
    else:
        nc = _build_nc_v8(lat_dt=lat_dt)
    if v10:
        # [D, TOK] -> [128, 16, TOK]: partition-major chunk layout
        lat_p = np.ascontiguousarray(
            latT_h.reshape(N_CHUNK, P, TOK).transpose(1, 0, 2))
    in_maps = []
    for c in range(N_CORES):
        in_maps.append({
            "latT": (np.ascontiguousarray(lat_p[:, :, c * TPC:(c + 1) * TPC])
                     if v10 else
                     np.ascontiguousarray(latT_h[:, c * TPC:(c + 1) * TPC])),
            "wt": wt_packed,
            "nzt": np.ascontiguousarray(nz_eff_t[:, c * TPC:(c + 1) * TPC]),
        })
    res = run_bass_kernel_spmd(nc, in_maps, core_ids=list(range(N_CORES)),
                               trace=_trace)
    out = np.empty((TOK, 3), dtype=np.float32)
    for c in range(N_CORES):
        out[c * TPC:(c + 1) * TPC] = res.results[c]["out"].T
    if _trace:
        kernel._last_results = res
    return out.reshape(B, S, 3)
